# revision 43
# baseline (speedup 1.0000x reference)
"""LGESQL line-graph GNN message-passing layer on 8 Trainium2 NeuronCores.

v5 design (edge-parallel, dst-sorted, all-bf16 data path):
  - Edges sorted by dst; 157 node-tiles of 128 dealt to 8 cores balanced by
    edge count (each core gets NSLOT=20 slot indices with a schedule that is
    identical across cores -> one SPMD program).
  - Phase 1: every core computes the full kv table [EP, 512] bf16 (k|v) with
    bf16 matmuls (4x faster than fp32 on PE) plus its local q table
    [2560, 256] bf16, both written to DRAM.
  - Phase 2 (per slot, per <=512-edge batch): one dma_gather pulls kv rows by
    src, a second pulls q rows by local dst.  Scores = per-head dot on DVE
    (bf16 2x elementwise + pairwise-fold tree + short 1x reduce), then ACT
    does exp with a broadcast read that also replicates scores across the
    head dim.  sv = v * scm on DVE; per-128-edge one-hot A (iota is_equal)
    feeds a PE matmul that segment-sums [wv | z] into PSUM per slot.
  - Phase 3: o = wv/z, output projection + LN, FFN via transposed-weight
    matmuls (f1 produced transposed so no 1024-wide transposes), + LN.
Host does index prep only; zero-valued biases/unit gains are skipped (flags
are part of the program cache key).
"""

import math
import os
SAFE = int(os.environ.get('KERNEL_SAFE', '29'))  # bitmask: 1=gather 2=exprep 4=rsqrt 8=ttr 16=actid (31=all safe)

import numpy as np

E = 20000
LE = 320000
NDIM = 256
P = 128
NCORES = 8
NT = (E + P - 1) // P            # 157 node tiles
EP = NT * P                      # 20096
NSLOT = (NT + NCORES - 1) // NCORES   # 20 slot indices per core
NROW = NSLOT * P                 # 2560 local nodes per core
UMAX = 9                         # max 128-edge units per gather batch
ISC = 1.0 / math.sqrt(32.0)

_CACHE = {}


# ----------------------------------------------------------------- host prep
def _prep(lg_src, lg_dst):
    lg_src = np.asarray(lg_src).astype(np.int64)
    lg_dst = np.asarray(lg_dst).astype(np.int64)
    deg = np.bincount(lg_dst, minlength=E)
    eorder = np.argsort(lg_dst, kind="stable")
    src_sorted = lg_src[eorder].astype(np.int32)
    row_start = np.zeros(E + 1, np.int64)
    row_start[1:] = np.cumsum(deg)

    tcnt = np.array([int(row_start[min((t + 1) * P, E)] - row_start[t * P])
                     for t in range(NT)])
    lo_cnt = np.zeros(NT, np.int64)
    for t in range(NT):
        a, b = int(row_start[t * P]), int(row_start[min((t + 1) * P, E)])
        lo_cnt[t] = int((src_sorted[a:b] < HALF).sum())
    torder = np.argsort(-lo_cnt, kind="stable")

    # slot s of core c <- tile torder[s*8+c] (or dummy)
    slot_tile = np.full((NSLOT, NCORES), -1, np.int64)
    for i, t in enumerate(torder):
        slot_tile[i // NCORES, i % NCORES] = t

    units = []
    batches = []
    for s in range(NSLOT):
        cnts = [tcnt[t] if t >= 0 else 0 for t in slot_tile[s]]
        u = max(1, int(math.ceil(max(cnts) / P)))
        units.append(u)
        bl = [UMAX] * (u // UMAX)
        if u % UMAX:
            bl.append(u % UMAX)
        batches.append(tuple(bl))
    units = tuple(units)
    batches = tuple(batches)
    ut_tot = sum(units)
    sumb = ut_tot * P

    per_core = []
    for c in range(NCORES):
        node_ids = np.zeros(NROW, np.int64)
        valid = np.zeros(NROW, bool)
        src_idx = np.zeros(sumb, np.int32)
        q_idx = np.zeros(sumb, np.int32)
        dstl = np.full((P, ut_tot), 255.0, np.float32)
        eoff = 0
        uoff = 0
        for s in range(NSLOT):
            t = slot_tile[s, c]
            lo = 0 if t < 0 else t * P
            n_real = 0 if t < 0 else max(0, min(P, E - lo))
            if n_real > 0:
                node_ids[s * P:s * P + n_real] = np.arange(lo, lo + n_real)
                valid[s * P:s * P + n_real] = True
            B = units[s] * P
            sidx = np.zeros(B, np.int16)
            qidx = np.full(B, s * P, np.int16)
            dl = np.full(B, 255.0, np.float32)
            if t >= 0 and n_real > 0:
                e0 = int(row_start[lo])
                e1 = int(row_start[lo + n_real])
                ne = e1 - e0
                sidx[:ne] = src_sorted[e0:e1].astype(np.int16)
                ldst = (lg_dst[eorder[e0:e1]] - lo).astype(np.int16)
                qidx[:ne] = (s * P + ldst).astype(np.int16)
                dl[:ne] = ldst.astype(np.float32)
            src_idx[eoff:eoff + B] = sidx
            q_idx[eoff:eoff + B] = qidx
            dstl[:, uoff:uoff + units[s]] = dl.reshape(units[s], P).T
            eoff += B
            uoff += units[s]

        def wrap(idx):
            # per-batch [16, B//16] blocks (idx i -> [i%16, i//16]), concat
            blocks = []
            off = 0
            for s in range(NSLOT):
                for ub in batches[s]:
                    B = ub * P
                    blocks.append(idx[off:off + B].reshape(-1, 16).T)
                    off += B
            w = np.concatenate(blocks, axis=1)
            return np.ascontiguousarray(np.tile(w, (8, 1)))

        per_core.append(dict(node_ids=node_ids, valid=valid,
                             src_w=wrap(src_idx), q_w=wrap(q_idx),
                             dstl=dstl))
    sched = dict(units=units, batches=batches, ut_tot=ut_tot, sumb=sumb)
    return sched, per_core


# ------------------------------------------------------------- device program
def _build(sched, flags, safe=0):
    import concourse.bacc as bacc
    import concourse.bass as bass
    import concourse.mybir as mybir
    import concourse.tile as tile
    from concourse.masks import make_identity
    from contextlib import ExitStack

    f32 = mybir.dt.float32
    bf16 = mybir.dt.bfloat16
    i16 = mybir.dt.int16
    AF = mybir.ActivationFunctionType
    OP = mybir.AluOpType
    AX = mybir.AxisListType.X

    units = sched["units"]
    batches = sched["batches"]
    ut_tot = sched["ut_tot"]
    sumb = sched["sumb"]
    g1_one, b1ln_zero, g2_one, b2ln_zero, b1_zero, b2_zero = flags

    nc = bacc.Bacc("TRN2", target_bir_lowering=False, debug=False)
    xT_d = nc.dram_tensor("xT", [256, EP], bf16, kind="ExternalInput")
    dstx_d = nc.dram_tensor("dstx", [256, EP], bf16, kind="ExternalInput")
    xTp_d = nc.dram_tensor("xTp", [256, NROW], bf16, kind="ExternalInput")
    sxq_d = nc.dram_tensor("sxq", [NROW, 256], bf16, kind="ExternalInput")
    rs1_d = nc.dram_tensor("rs1", [NROW, 256], bf16, kind="ExternalInput")
    wq_d = nc.dram_tensor("wq", [P, 512], bf16, kind="ExternalInput")
    wk_d = nc.dram_tensor("wk", [P, 512], bf16, kind="ExternalInput")
    wv_d = nc.dram_tensor("wv", [P, 512], bf16, kind="ExternalInput")
    wo_d = nc.dram_tensor("wo", [P, 512], bf16, kind="ExternalInput")
    w1_d = nc.dram_tensor("w1", [P, 2048], bf16, kind="ExternalInput")
    w2_d = nc.dram_tensor("w2", [P, 2048], bf16, kind="ExternalInput")
    iot_d = nc.dram_tensor("iot", [P, P], bf16, kind="ExternalInput")
    sidx_d = nc.dram_tensor("sidx", [P, ut_tot], mybir.dt.int32,
                            kind="ExternalInput")
    qidx_d = nc.dram_tensor("qidx", [P, ut_tot], mybir.dt.int32,
                            kind="ExternalInput")
    dstl_d = nc.dram_tensor("dstl", [P, ut_tot], f32, kind="ExternalInput")
    opt_d = {}
    for nm, skip in [("lng1", g1_one), ("lnb1", b1ln_zero),
                     ("lng2", g2_one), ("lnb2", b2ln_zero)]:
        if not skip:
            opt_d[nm] = nc.dram_tensor(nm, [P, 256], bf16,
                                       kind="ExternalInput")
    if not b1_zero:
        opt_d["b1r"] = nc.dram_tensor("b1r", [P, 8], bf16,
                                      kind="ExternalInput")
    if not b2_zero:
        opt_d["b2r"] = nc.dram_tensor("b2r", [P, 256], bf16,
                                      kind="ExternalInput")
    out_d = nc.dram_tensor("out", [NROW, 256], f32, kind="ExternalOutput")

    with tile.TileContext(nc) as tc:
        with ExitStack() as ctx:
            cst = ctx.enter_context(tc.tile_pool(name="cst", bufs=1))
            drm = ctx.enter_context(tc.tile_pool(name="drm", bufs=1,
                                                 space="DRAM"))
            kv = drm.tile([EP, 512], bf16)
            qloc = drm.tile([NROW, 256], bf16)

            def load_const(dram, shape, dtype=bf16):
                t = cst.tile(shape, dtype, tag=dram.name + "_c")
                nc.sync.dma_start(out=t[:], in_=dram[:])
                return t

            wq_s = load_const(wq_d, [P, 512])
            wk_s = load_const(wk_d, [P, 512])
            wv_s = load_const(wv_d, [P, 512])
            iota = load_const(iot_d, [P, P])
            sidx_s = load_const(sidx_d, [P, ut_tot], mybir.dt.int32)
            qidx_s = load_const(qidx_d, [P, ut_tot], mybir.dt.int32)
            dstl_s = load_const(dstl_d, [P, ut_tot], f32)
            opt_s = {nm: load_const(d, list(d.shape))
                     for nm, d in opt_d.items()}
            ident = cst.tile([P, P], bf16)
            make_identity(nc, ident[:])
            wvsb = cst.tile([P, NSLOT, 264], bf16)
            ident2 = cst.tile([P, 512], bf16)
            nc.vector.memset(ident2[:], 0.0)
            nc.vector.tensor_copy(out=ident2[:, 0:P], in_=ident[:])
            nc.vector.tensor_copy(out=ident2[:, 256 + P:512], in_=ident[:, 0:P])
            cvals = cst.tile([P, 2], f32)
            nc.vector.memset(cvals[:, 0:1], 0.0)
            nc.vector.memset(cvals[:, 1:2], 1e-5)
            nc.const_aps.aps[(f32, 0.0)] = cvals[:, 0:1]
            nc.const_aps.aps[(f32, 1e-5)] = cvals[:, 1:2]


            # ---------------- phase 1: kv + q tables ----------------
            EG = 2
            gat = ctx.enter_context(tc.tile_pool(name="gat", bufs=3))
            sco = ctx.enter_context(tc.tile_pool(name="sco", bufs=3))
            svp = ctx.enter_context(tc.tile_pool(name="svp", bufs=3))
            p3 = ctx.enter_context(tc.tile_pool(name="p3", bufs=2))
            wvps = ctx.enter_context(
                tc.tile_pool(name="wvps", bufs=2, space="PSUM"))
            psA = ctx.enter_context(
                tc.tile_pool(name="psA", bufs=3, space="PSUM"))
            p3ps = psA
            p1sb = ctx.enter_context(tc.tile_pool(name="p1sb", bufs=3))
            p1ps = psA
            if True:
                for g in range((NT + EG - 1) // EG):
                    t0 = g * EG
                    ntl = min(EG, NT - t0)
                    rows = ntl * P
                    r0 = t0 * P
                    xt_g = p1sb.tile([P, 2, EG * P], bf16, tag="xtg")
                    nc.sync.dma_start(
                        out=xt_g[:, :, :rows],
                        in_=xT_d[:, r0:r0 + rows].rearrange(
                            "(c p) n -> p c n", p=P))
                    dx_g = p1sb.tile([P, 2, EG * P], bf16, tag="dxg")
                    nc.sync.dma_start(
                        out=dx_g[:, :, :rows],
                        in_=dstx_d[:, r0:r0 + rows].rearrange(
                            "(c p) n -> p c n", p=P))
                    kv_ps = p1ps.tile([P, EG, 512], f32, tag="ps")
                    for i in range(ntl):
                        for kk in range(2):
                            nc.tensor.matmul(
                                kv_ps[:, i, 0:256],
                                xt_g[:, kk, i * P:(i + 1) * P],
                                wk_s[:, kk * 256:(kk + 1) * 256],
                                start=(kk == 0), stop=(kk == 1))
                        for kk in range(2):
                            nc.tensor.matmul(
                                kv_ps[:, i, 256:512],
                                xt_g[:, kk, i * P:(i + 1) * P],
                                wv_s[:, kk * 256:(kk + 1) * 256],
                                start=(kk == 0), stop=False)
                        for kk in range(2):
                            nc.tensor.matmul(
                                kv_ps[:, i, 256:512],
                                dx_g[:, kk, i * P:(i + 1) * P],
                                ident2[:, kk * 256:(kk + 1) * 256],
                                start=False, stop=(kk == 1))
                    kv_sb = p1sb.tile([P, EG, 512], bf16, tag="kvsb")
                    nc.scalar.activation(kv_sb[:, :ntl, :],
                                         kv_ps[:, :ntl, :], AF.Copy)
                    nc.sync.dma_start(
                        out=kv[r0:r0 + rows, :].rearrange(
                            "(t p) n -> p t n", p=P),
                        in_=kv_sb[:, :ntl, :])

                for s in range(NSLOT):
                    xp_t = p1sb.tile([P, 2, P], bf16, tag="xpt")
                    nc.sync.dma_start(
                        out=xp_t[:],
                        in_=xTp_d[:, s * P:(s + 1) * P].rearrange(
                            "(c p) n -> p c n", p=P))
                    q_ps = p1ps.tile([P, 256], f32, tag="ps")
                    for kk in range(2):
                        nc.tensor.matmul(q_ps[:], xp_t[:, kk, :],
                                         wq_s[:, kk * 256:(kk + 1) * 256],
                                         start=(kk == 0), stop=(kk == 1))
                    sxq_t = p1sb.tile([P, 256], bf16, tag="sxqt")
                    nc.sync.dma_start(out=sxq_t[:],
                                      in_=sxq_d[s * P:(s + 1) * P, :])
                    q_sb = p1sb.tile([P, 256], bf16, tag="qsb")
                    nc.vector.tensor_tensor(out=q_sb[:], in0=q_ps[:],
                                            in1=sxq_t[:], op=OP.add)
                    nc.sync.dma_start(out=qloc[s * P:(s + 1) * P, :],
                                      in_=q_sb[:])

            # ---------------- phases 2+3 ----------------
            if True:

                def rsqrt_chain_safe(vh4, rstd4, scr4):
                    # vh4 = (var+eps)/2 -> rstd = 1/sqrt(2*vh4)
                    nc.vector.tensor_scalar(out=scr4, in0=vh4, scalar1=2.0,
                                            scalar2=None, op0=OP.mult)
                    nc.scalar.activation(rstd4, scr4, AF.Sqrt)
                    nc.vector.reciprocal(rstd4, rstd4)

                def rsqrt_chain(vh4, rstd4, scr4):
                    if safe & 4:
                        return rsqrt_chain_safe(vh4, rstd4, scr4)
                    # rstd4 = rsqrt(2*vh4) elementwise on [P, W] via DVE
                    # bit-trick seed (magic adjusted for the v/2 input) plus
                    # two Newton iterations y' = y*(1.5 - vh*y*y).
                    iv = rstd4.bitcast(mybir.dt.int32)
                    nc.vector.tensor_scalar(
                        out=iv, in0=vh4.bitcast(mybir.dt.int32),
                        scalar1=1, scalar2=-1,
                        op0=OP.logical_shift_right, op1=OP.bitwise_xor)
                    nc.vector.tensor_scalar(
                        out=iv, in0=iv, scalar1=0x5EF759E0, scalar2=None,
                        op0=OP.add)
                    for _ in range(2):
                        nc.vector.tensor_tensor(out=scr4, in0=rstd4,
                                                in1=rstd4, op=OP.mult)
                        nc.vector.tensor_tensor(out=scr4, in0=scr4,
                                                in1=vh4, op=OP.mult)
                        nc.vector.tensor_scalar(
                            out=scr4, in0=scr4, scalar1=-1.0,
                            scalar2=1.5, op0=OP.mult, op1=OP.add)
                        nc.vector.tensor_tensor(out=rstd4, in0=rstd4,
                                                in1=scr4, op=OP.mult)

                def ln_stats(h_c, sum_ap, vh_ap, tmp):
                    nc.vector.tensor_scalar(out=tmp[:, 0:1], in0=sum_ap,
                                            scalar1=-1.0 / 256, scalar2=None,
                                            op0=OP.mult)
                    if safe & 8:
                        nc.vector.tensor_scalar(out=h_c, in0=h_c,
                                                scalar1=tmp[:, 0:1],
                                                scalar2=None, op0=OP.add)
                        sq = sco.tile([P, 256], f32, tag="lnsq")
                        nc.vector.tensor_tensor(out=sq[:], in0=h_c, in1=h_c,
                                                op=OP.mult)
                        nc.vector.tensor_reduce(out=tmp[:, 2:3], in_=sq[:],
                                                axis=AX, op=OP.add)
                        nc.vector.tensor_scalar(out=vh_ap, in0=tmp[:, 2:3],
                                                scalar1=0.5 / 256,
                                                scalar2=0.5e-5, op0=OP.mult,
                                                op1=OP.add)
                        return
                    nc.scalar.activation(h_c, h_c, AF.Identity,
                                         bias=tmp[:, 0:1])
                    sq = sco.tile([P, 256], f32, tag="lnsq")
                    nc.vector.tensor_tensor_reduce(
                        out=sq[:], in0=h_c, in1=h_c, scale=0.5 / 256,
                        scalar=0.5e-5, op0=OP.mult, op1=OP.add,
                        accum_out=vh_ap)

                def ln_apply(h_c, rstd_col, out_ap, g_t, b_t):
                    if g_t is None:
                        if safe & 16:
                            nc.vector.tensor_scalar(out=out_ap, in0=h_c,
                                                    scalar1=rstd_col,
                                                    scalar2=None, op0=OP.mult)
                        else:
                            nc.scalar.activation(out_ap, h_c, AF.Identity,
                                                 scale=rstd_col)
                    else:
                        nc.vector.scalar_tensor_tensor(
                            out=out_ap, in0=h_c, scalar=rstd_col,
                            in1=g_t[:], op0=OP.mult, op1=OP.mult)
                    if b_t is not None:
                        nc.vector.tensor_tensor(out=out_ap, in0=out_ap,
                                                in1=b_t[:], op=OP.add)

                # unit offsets per (slot, epoch)
                uoff_tab = {}
                uo_ = 0
                for s in range(NSLOT):
                    for ep in range(2):
                        uoff_tab[(s, ep)] = uo_
                        uo_ += units[s][ep]

                def do_epoch(s, ep, wv_ps, stop_at_end):
                    uoff = uoff_tab[(s, ep)]
                    kv_src = kv_lo if ep == 0 else kv_hi
                    nbat = len(batches[s][ep])
                    for bi, ub in enumerate(batches[s][ep]):
                        B = ub * P
                        kv_g = gat.tile([P, UMAX, 512], bf16, tag="kvg")
                        q_g = gat.tile([P, UMAX, 256], bf16, tag="qg")
                        if safe & 1:
                            for j in range(ub):
                                nc.gpsimd.indirect_dma_start(
                                    out=kv_g[:, j, :], out_offset=None,
                                    in_=kv_src[:, :],
                                    in_offset=bass.IndirectOffsetOnAxis(
                                        ap=sidx_s[:, uoff + j:uoff + j + 1],
                                        axis=0))
                                nc.gpsimd.indirect_dma_start(
                                    out=q_g[:, j, :], out_offset=None,
                                    in_=qloc[:, :],
                                    in_offset=bass.IndirectOffsetOnAxis(
                                        ap=qidx_s[:, uoff + j:uoff + j + 1],
                                        axis=0))
                        else:
                            nc.gpsimd.indirect_dma_start(
                                out=kv_g[:, :ub, :], out_offset=None,
                                in_=kv_src[:, :],
                                in_offset=bass.IndirectOffsetOnAxis(
                                    ap=sidx_s[:, uoff:uoff + ub], axis=0))
                            nc.gpsimd.indirect_dma_start(
                                out=q_g[:, :ub, :], out_offset=None,
                                in_=qloc[:, :],
                                in_offset=bass.IndirectOffsetOnAxis(
                                    ap=qidx_s[:, uoff:uoff + ub], axis=0))
                        prodk = sco.tile([P, UMAX, 256], bf16, tag="prodk")
                        nc.vector.tensor_tensor(
                            out=prodk[:, :ub, :], in0=kv_g[:, :ub, 0:256],
                            in1=q_g[:, :ub, :], op=OP.mult)
                        pk4 = prodk.rearrange("p u (h d) -> p u h d", d=32)
                        fa = sco.tile([P, UMAX, 8, 16], bf16, tag="fa")
                        nc.vector.tensor_tensor(
                            out=fa[:, :ub], in0=pk4[:, :ub, :, 0:16],
                            in1=pk4[:, :ub, :, 16:32], op=OP.add)
                        fb = sco.tile([P, UMAX, 8, 8], bf16, tag="fb")
                        nc.vector.tensor_tensor(
                            out=fb[:, :ub], in0=fa[:, :ub, :, 0:8],
                            in1=fa[:, :ub, :, 8:16], op=OP.add)
                        fc = sco.tile([P, UMAX, 8, 4], bf16, tag="fc")
                        nc.vector.tensor_tensor(
                            out=fc[:, :ub], in0=fb[:, :ub, :, 0:4],
                            in1=fb[:, :ub, :, 4:8], op=OP.add)
                        scr = sco.tile([P, UMAX, 8], f32, tag="scr")
                        nc.vector.tensor_reduce(out=scr[:, :ub],
                                                in_=fc[:, :ub],
                                                axis=AX, op=OP.add)
                        sv = svp.tile([P, UMAX, 264], bf16, tag="sv")
                        if safe & 2:
                            scm8 = sco.tile([P, UMAX, 8], bf16, tag="scm8")
                            nc.scalar.activation(scm8[:, :ub], scr[:, :ub],
                                                 AF.Exp, scale=ISC)
                            nc.vector.tensor_copy(out=sv[:, :ub, 256:264],
                                                  in_=scm8[:, :ub])
                            nc.vector.tensor_tensor(
                                out=sv[:, :ub, 0:256].rearrange(
                                    "p u (h d) -> p u h d", d=32),
                                in0=kv_g[:, :ub, 256:512].rearrange(
                                    "p u (h d) -> p u h d", d=32),
                                in1=scm8[:, :ub].unsqueeze(3).to_broadcast(
                                    [P, ub, 8, 32]),
                                op=OP.mult)
                        else:
                            scm = sco.tile([P, UMAX, 256], bf16, tag="scm")
                            nc.scalar.activation(
                                scm[:, :ub].rearrange("p u (h d) -> p u h d",
                                                      d=32),
                                scr[:, :ub].unsqueeze(3).to_broadcast(
                                    [P, ub, 8, 32]),
                                AF.Exp, scale=ISC)
                            nc.scalar.activation(sv[:, :ub, 256:264],
                                                 scr[:, :ub], AF.Exp,
                                                 scale=ISC)
                            nc.vector.tensor_tensor(
                                out=sv[:, :ub, 0:256],
                                in0=kv_g[:, :ub, 256:512],
                                in1=scm[:, :ub, :], op=OP.mult)
                        A_t = svp.tile([P, UMAX, P], bf16, tag="At")
                        for j in range(ub):
                            nc.vector.tensor_scalar(
                                out=A_t[:, j, :], in0=iota[:],
                                scalar1=dstl_s[:, uoff + j:uoff + j + 1],
                                scalar2=None, op0=OP.is_equal)
                        for j in range(ub):
                            nc.tensor.matmul(
                                wv_ps[:], A_t[:, j, :], sv[:, j, :],
                                start=(bi == 0 and j == 0),
                                stop=(stop_at_end and bi == nbat - 1
                                      and j == ub - 1))

                grp = {}
                # sweep A: epoch-0 partials for every slot -> SBUF
                for s in range(NSLOT):
                    wv_ps = wvps.tile([P, 264], f32, tag="wv")
                    do_epoch(s, 0, wv_ps, True)
                    nc.scalar.activation(wvsb[:, s, :], wv_ps[:], AF.Copy)
                # sweep B: epoch-1 + phase 3
                for s in range(NSLOT):
                    wv_ps = wvps.tile([P, 264], f32, tag="wv")
                    do_epoch(s, 1, wv_ps, True)
                    # ---- phase 3a for slot s ----
                    tmp = p3.tile([P, 8], f32, tag="tmp")
                    zr = p3.tile([P, 8], f32, tag="zr")
                    nc.vector.tensor_tensor(out=zr[:], in0=wv_ps[:, 256:264],
                                            in1=wvsb[:, s, 256:264],
                                            op=OP.add)
                    nc.vector.tensor_scalar(out=zr[:], in0=zr[:],
                                            scalar1=1e-30, scalar2=None,
                                            op0=OP.add)
                    nc.vector.reciprocal(zr[:], zr[:])
                    wvf = p3.tile([P, 256], f32, tag="wvf")
                    nc.vector.tensor_tensor(out=wvf[:], in0=wv_ps[:, 0:256],
                                            in1=wvsb[:, s, 0:256], op=OP.add)
                    o_sb = p3.tile([P, 256], bf16, tag="osb")
                    nc.vector.tensor_tensor(
                        out=o_sb[:].rearrange("p (h d) -> p h d", d=32),
                        in0=wvf[:].rearrange("p (h d) -> p h d", d=32),
                        in1=zr[:].unsqueeze(2).to_broadcast([P, 8, 32]),
                        op=OP.mult)
                    oT = p3.tile([P, 2, P], bf16, tag="oT")
                    for cc in range(2):
                        tp = p3ps.tile([P, P], bf16, tag="ps")
                        nc.tensor.transpose(tp[:],
                                            o_sb[:, cc * P:(cc + 1) * P],
                                            ident[:])
                        nc.scalar.activation(oT[:, cc, :], tp[:], AF.Copy)
                    h_ps = p3ps.tile([P, 256], f32, tag="ps")
                    for kk in range(2):
                        nc.tensor.matmul(h_ps[:], oT[:, kk, :],
                                         wo_s[:, kk * 256:(kk + 1) * 256],
                                         start=(kk == 0), stop=(kk == 1))
                    k4 = s % 4
                    if k4 == 0:
                        grp["hc4"] = p3.tile([P, 4, 256], bf16, tag="hc4",
                                             name="hc4")
                        grp["vh4"] = p3.tile([P, 4], f32, tag="vh4",
                                             name="vh4")
                        grp["h_bf"] = p3.tile([P, 4, 256], bf16, tag="hbf4",
                                              name="hbf4")
                        grp["hT"] = p3.tile([P, 2, 4, P], bf16, tag="hT4",
                                            name="hT4")
                    rs1_t = p3.tile([P, 256], bf16, tag="rs1t")
                    nc.sync.dma_start(out=rs1_t[:],
                                      in_=rs1_d[s * P:(s + 1) * P, :])
                    h_c = grp["hc4"][:, k4, :]
                    if safe & 8:
                        nc.vector.tensor_tensor(out=h_c, in0=h_ps[:],
                                                in1=rs1_t[:], op=OP.add)
                        nc.vector.tensor_reduce(out=tmp[:, 1:2], in_=h_c,
                                                axis=AX, op=OP.add)
                    else:
                        nc.vector.tensor_tensor_reduce(
                            out=h_c, in0=h_ps[:], in1=rs1_t[:], scale=1.0,
                            scalar=0.0, op0=OP.add, op1=OP.add,
                            accum_out=tmp[:, 1:2])
                    ln_stats(h_c, tmp[:, 1:2], grp["vh4"][:, k4:k4 + 1], tmp)

                    if k4 == 3:
                        s0 = s - 3
                        rstd4 = p3.tile([P, 4], f32, tag="rstd4")
                        scr4 = p3.tile([P, 4], f32, tag="scr4")
                        rsqrt_chain(grp["vh4"][:], rstd4[:], scr4[:])
                        for k in range(4):
                            ln_apply(grp["hc4"][:, k, :], rstd4[:, k:k + 1],
                                     grp["h_bf"][:, k, :],
                                     opt_s.get("lng1"), opt_s.get("lnb1"))
                            tp = p3ps.tile([P, 2, P], bf16, tag="ps")
                            for cc in range(2):
                                nc.tensor.transpose(
                                    tp[:, cc, :],
                                    grp["h_bf"][:, k, cc * P:(cc + 1) * P],
                                    ident[:])
                            nc.scalar.activation(
                                grp["hT"][:, :, k, :], tp[:], AF.Copy)
                        f1_sb = p3.tile([P, 8, 512], bf16, tag="f1sb")
                        for uo in range(8):
                            f1_ps = p3ps.tile([P, 512], f32, tag="ps")
                            for cc in range(2):
                                nc.tensor.matmul(
                                    f1_ps[:],
                                    w1_s[:, cc * 1024 + uo * P:
                                         cc * 1024 + (uo + 1) * P],
                                    grp["hT"][:, cc, :, :].rearrange(
                                        "p k n -> p (k n)"),
                                    start=(cc == 0), stop=(cc == 1))
                            if not b1_zero:
                                nc.vector.tensor_scalar(
                                    out=f1_ps[:], in0=f1_ps[:],
                                    scalar1=opt_s["b1r"][:, uo:uo + 1],
                                    scalar2=None, op0=OP.add)
                            nc.scalar.activation(f1_sb[:, uo, :], f1_ps[:],
                                                 AF.Relu)
                        hc2 = p3.tile([P, 4, 256], bf16, tag="hc2")
                        vh2 = p3.tile([P, 4], f32, tag="vh2")
                        for k in range(4):
                            h2_ps = p3ps.tile([P, 256], f32, tag="ps")
                            for uo in range(8):
                                nc.tensor.matmul(
                                    h2_ps[:],
                                    f1_sb[:, uo, k * P:(k + 1) * P],
                                    w2_s[:, uo * 256:(uo + 1) * 256],
                                    start=(uo == 0), stop=(uo == 7))
                            tmp2 = p3.tile([P, 8], f32, tag="tmp2")
                            h2c = hc2[:, k, :]
                            src2 = h2_ps[:]
                            if not b2_zero:
                                nc.vector.tensor_tensor(
                                    out=h2c, in0=h2_ps[:],
                                    in1=opt_s["b2r"][:], op=OP.add)
                                src2 = h2c
                            if safe & 8:
                                nc.vector.tensor_tensor(
                                    out=h2c, in0=src2,
                                    in1=grp["h_bf"][:, k, :], op=OP.add)
                                nc.vector.tensor_reduce(
                                    out=tmp2[:, 1:2], in_=h2c, axis=AX,
                                    op=OP.add)
                            else:
                                nc.vector.tensor_tensor_reduce(
                                    out=h2c, in0=src2,
                                    in1=grp["h_bf"][:, k, :], scale=1.0,
                                    scalar=0.0, op0=OP.add, op1=OP.add,
                                    accum_out=tmp2[:, 1:2])
                            ln_stats(h2c, tmp2[:, 1:2], vh2[:, k:k + 1],
                                     tmp2)
                        rstd2 = p3.tile([P, 4], f32, tag="rstd2")
                        scr2 = p3.tile([P, 4], f32, tag="scr2")
                        rsqrt_chain(vh2[:], rstd2[:], scr2[:])
                        for k in range(4):
                            out_sb = p3.tile([P, 256], f32, tag="outsb")
                            ln_apply(hc2[:, k, :], rstd2[:, k:k + 1],
                                     out_sb[:], opt_s.get("lng2"),
                                     opt_s.get("lnb2"))
                            nc.sync.dma_start(
                                out=out_d[(s0 + k) * P:(s0 + k + 1) * P, :],
                                in_=out_sb[:])
    nc.compile()
    return nc


# ------------------------------------------------------------------- host
def _host_arrays(x, src_x, dst_x, Wq, bq, Wk, Wv, Wo, bo, ln1_g, ln1_b,
                 W1, b1, W2, b2, ln2_g, ln2_b, lg_src, lg_dst):
    import ml_dtypes
    bf = ml_dtypes.bfloat16

    x = np.asarray(x, np.float32)
    src_x = np.asarray(src_x, np.float32)
    dst_x = np.asarray(dst_x, np.float32)
    bq = np.asarray(bq, np.float32)
    bo = np.asarray(bo, np.float32)
    sched, per_core = _prep(lg_src, lg_dst)

    flags = (bool(np.all(np.asarray(ln1_g) == 1)),
             bool(np.all(np.asarray(ln1_b) == 0)),
             bool(np.all(np.asarray(ln2_g) == 1)),
             bool(np.all(np.asarray(ln2_b) == 0)),
             bool(np.all(np.asarray(b1) == 0)),
             bool(np.all(np.asarray(b2) == 0)))

    def wlayout(w, nchunk):
        w = np.asarray(w, np.float32)
        k, n = w.shape
        return np.ascontiguousarray(
            w.reshape(nchunk, P, n).transpose(1, 0, 2)
            .reshape(P, nchunk * n)).astype(bf)

    rep = lambda v: np.ascontiguousarray(
        np.tile(np.asarray(v, np.float32)[None, :], (P, 1))).astype(bf)

    xp = np.zeros((EP, 256), np.float32)
    xp[:E] = x
    dxp = np.zeros((EP, 256), np.float32)
    dxp[:E] = dst_x

    shared = dict(
        xT=np.ascontiguousarray(xp.T).astype(bf),
        dstx=np.ascontiguousarray(dxp.T).astype(bf),
        wq=wlayout(Wq, 2), wk=wlayout(Wk, 2), wv=wlayout(Wv, 2),
        wo=wlayout(Wo, 2), w1=wlayout(W1, 2), w2=wlayout(W2, 8),
        iot=np.ascontiguousarray(
            np.tile(np.arange(P, dtype=np.float32), (P, 1))).astype(bf),
    )
    g1, b1ln, g2, b2ln, b1z, b2z = flags
    if not g1:
        shared["lng1"] = rep(ln1_g)
    if not b1ln:
        shared["lnb1"] = rep(ln1_b)
    if not g2:
        shared["lng2"] = rep(ln2_g)
    if not b2ln:
        shared["lnb2"] = rep(ln2_b)
    if not b1z:
        shared["b1r"] = np.ascontiguousarray(
            np.asarray(b1, np.float32).reshape(8, P).T).astype(bf)
    if not b2z:
        shared["b2r"] = rep(b2)

    in_maps = []
    for c in range(NCORES):
        pc = per_core[c]
        ids = pc["node_ids"]
        in_maps.append(dict(
            shared,
            xTp=np.ascontiguousarray(x[ids].T).astype(bf),
            sxq=np.ascontiguousarray(src_x[ids] + bq[None, :]).astype(bf),
            rs1=np.ascontiguousarray(x[ids] + bo[None, :]).astype(bf),
            sidx=pc["src_w"],
            qidx=pc["q_w"],
            dstl=pc["dstl"],
        ))
    return sched, per_core, flags, in_maps


def kernel(x, src_x, dst_x, Wq, bq, Wk, Wv, Wo, bo, ln1_g, ln1_b,
           W1, b1, W2, b2, ln2_g, ln2_b, lg_src, lg_dst):
    from concourse.bass_utils import run_bass_kernel_spmd

    sched, per_core, flags, in_maps = _host_arrays(
        x, src_x, dst_x, Wq, bq, Wk, Wv, Wo, bo, ln1_g, ln1_b,
        W1, b1, W2, b2, ln2_g, ln2_b, lg_src, lg_dst)

    key = (sched["units"], sched["batches"], flags)
    if key not in _CACHE:
        _CACHE[key] = _build(sched, flags, SAFE)
    nc = _CACHE[key]

    trace = bool(int(os.environ.get("KERNEL_TRACE", "0")))
    res = run_bass_kernel_spmd(nc, in_maps, list(range(NCORES)),
                               trace=trace)
    global LAST_EXEC_NS, LAST_RESULTS
    LAST_EXEC_NS = res.exec_time_ns
    LAST_RESULTS = res

    out = np.zeros((E, 256), np.float32)
    for c in range(NCORES):
        pc = per_core[c]
        o = np.asarray(res.results[c]["out"])
        v = pc["valid"]
        out[pc["node_ids"][v]] = o[v]
    return out


LAST_EXEC_NS = None
LAST_RESULTS = None
